# revision 26
# baseline (speedup 1.0000x reference)
"""Trainium2 Bass kernel for the BiaffineLayer problem.

Math (per batch b):
  out[l, m, c] = x1[l] @ W1[c] + x2[m] @ W2[c]
              + sum_h x1[l,h] * x2[m,h] * W3[c,h]
              + sum_h |x1[l,h] - x2[m,h]| * W4[c,h] + bias[c]
  shapes: x1, x2 [2, 512, 128]; W [25, 512]; bias [25]; out [2, 512, 512, 25]

Sharding: 8 cores = 2 batches x 4 m-blocks of 128 columns. Core (b, mb) gets
full x1[b] and its x2[b, m0:m0+128] block; it produces out[b, :, m0:m0+MB, :].

Decomposition, with |d| = 2*relu(d) - d and d = x1 - x2[m]:
  out = x1t' @ V3  +  D_m' @ (2 W4T)  +  T2B
where (host-precomputed except D):
  V3[h,(m,c)] = x2[m,h]*W3[c,h] + (W1-W4)[c,h]   (t3 + t1 - t4's -x1*W4 part)
  T2B[m,c]    = x2[m] @ (W2+W4)T + b             (added on the HOST during
                                                  unshard - pure per-(m,c))
  D_m[h,l]    = relu(x1[l,h] - x2[m,h])          (device, pairwise)

Measured HW rates (microbenched this session):
  - DVE tensor_scalar = ~129ns fixed + 0.26ns/elem (263ns per [128,512]
    D tile; the bf16 2x mode is already engaged; the f32 ptr scalar costs
    a fixed ~60ns; no 4x mode reachable). D-gen on DVE is THE critical
    resource: 108 tiles x 263ns = 28.4us.
  - ACT relu-with-bias ~710ns/tile in-kernel, ACT [128,800] PSUM drain
    ~930ns. ACT carries 20 D tiles + the drains and still has slack.
  - PE: t4 LDW+MM pair 25ns, t3 400-col MM 168ns; PE total ~18us, never
    critical. fp8 gains nothing (LDW is row-bound, DVE/ACT fp8-out is
    1.5x SLOWER, and e4m3 would not fit the error budget anyway).
  - DMA: ~0.7us issue + ~1.5us ring startup + ~17ns per partition row;
    only sync/scalar/gpsimd queues can issue; a ring's second DMA starts
    ~1us after its first; large out-DMAs on the gpsimd ring can hit a
    ~19us descriptor pathology (keep them on sync).
  - Framework overhead: ~7.3us prologue (engine barriers etc.) + ~3.4us
    counted teardown; both effectively fixed.

Schedule:
  - xin = x1t|negx2-bytes merged into one [H, 768] bf16 tensor so negx2
    rides x1t's fat 1536B rows (a separate 512B-row DMA measured ~5x
    slower and landed last); split by PARTITION halves (descriptor count
    scales with rows) on the sync+scalar rings, first-on-ring. w4t2+v3
    merged into wv3; its first chunk is first on the gpsimd ring, the
    bulk rides second there (lands ~15.5us, needed ~17.5us).
  - Per 16-m block: DVE makes 12-14 D tiles, ACT 2-4 (placed EARLY in j
    so the block's end never waits on the slower ACT; cadence is then
    pure DVE). Two 2-bank PSUM tiles per block (pool depth 2 blocks);
    t3 opens each group full-width, t4s accumulate j-major.
  - Drains (2 ACT copies + 1 sync out-DMA per block) emitted one block
    late so they run in ACT's idle tail. The final block is DVE-light
    (nV=12) and its drain splits DVE+ACT in PARALLEL - this requires the
    two halves to read SEPARATE psum tiles (cross-engine readers of one
    tile serialize in the framework) - with its two out-DMAs on the
    sync+scalar queues.
  - Output DRAM layout is block-major [ms, p, lc*c]: each partition's
    3200B is one contiguous DMA descriptor (vs 512x800B strided rows
    for an l-major layout), shortening the tail transfers.
  - Host converts bf16 output to f32, adds T2B, untangles the
    block-major layout, and reassembles [B, L, L, C].

Measured end-to-end (fresh device): ~45.2-45.6us. Anatomy: 7.3us fixed
prologue; D-gen starts ~10.0us (DMA ring startup+transfer); 28.45us
gapless DVE-bound region; PE stop +0.35; parallel drain halves +1.0;
final DMA issue+transfer+latency ~3.1; fixed barrier epilogue ~2.5.
"""

import sys

sys.path.insert(0, "/opt/trn_rl_repo")

from contextlib import ExitStack

import ml_dtypes
import numpy as np

import concourse.bass as bass
import concourse.tile as tile
from concourse import bacc, bass_utils, mybir

F32 = mybir.dt.float32
BF16 = mybir.dt.bfloat16
BF16_NP = ml_dtypes.bfloat16

B, L, H, C = 2, 512, 128, 25
MB = 128            # m-block per core
N_CORES = 8
MSUB = 16           # m's per psum block
N_MS = MB // MSUB   # 8 blocks over the m-block
LCHUNK = 128
N_LC = L // LCHUNK  # 4 l-chunks
CHUNK_F = MSUB * C  # 400 psum free columns per l-chunk slice
PS_STRIDE = 512     # psum bank stride (f32 elems) per l-chunk slice

# Per-block D-tile engine split: V=DVE, A=ACT. sum V = 108.
# ACT's per-block queue is [A-tiles..., drain(k-1)]; A positions are EARLY
# (j=1,3,..) so the block's end never waits on the slower ACT engine --
# the cadence is then set purely by DVE (nV x 263ns). The drain runs in
# ACT's idle tail of each block. First and last blocks are ACT-heavy:
# block 0 has no drain yet, and a DVE-light final block shortens the tail
# (DVE finishes early and takes half of the final drain).
D_PATS = {
    12: "VAVAVAVAVVVVVVVV",   # V=12 A=4 at j=1,3,5,7
    14: "VAVAVVVVVVVVVVVV",   # V=14 A=2 at j=1,3
}
D_NV = [12, 14, 14, 14, 14, 14, 14, 12]   # sum = 108


def build_kernel(nc: bass.Bass, repeat: int = 1):
    # xin = x1t bf16 [H, 512] cols 0:512 | negx2 f32 [H, 128] as bf16-viewed
    # bytes in cols 512:768. Merging rides negx2's bytes on x1t's fat
    # 1536B rows: one DMA per partition-half at full bandwidth instead of
    # a separate 512B-row DMA that measured ~5x slower and landed last.
    xin = nc.dram_tensor("xin", (H, L + 2 * MB), BF16, kind="ExternalInput").ap()
    # w4t2 (25 cols) | v3 (3200 cols) merged
    wv3 = nc.dram_tensor("wv3", (H, C + MB * C), BF16, kind="ExternalInput").ap()
    # block-major output [ms, p, lc*c]: each partition's 3200B is one
    # contiguous DMA descriptor (vs 512x800B for an l-major layout); the
    # host untangles the layout during unshard
    out = nc.dram_tensor("out", (N_MS, LCHUNK, N_LC * CHUNK_F), BF16,
                         kind="ExternalOutput").ap()

    WVA = C + 2 * CHUNK_F        # w4t2 + v3 blocks 0-1

    with tile.TileContext(nc) as tc, ExitStack() as ctx:
      const = ctx.enter_context(tc.tile_pool(name="const", bufs=1))
      dpool = ctx.enter_context(tc.tile_pool(name="dpool", bufs=128))
      opool = ctx.enter_context(tc.tile_pool(name="opool", bufs=10))
      psum = ctx.enter_context(tc.tile_pool(name="psum", bufs=4, space="PSUM"))
      for _rep in range(repeat):
        # ---- input loads ----
        # DMA queues are sync/scalar/gpsimd only. A DMA costs ~1.3us
        # fixed startup + transfer at ~230GB/s for >=1KB rows, and a
        # ring's SECOND DMA starts ~1us after its first finishes. So
        # everything D-gen needs rides FIRST on a ring: xin halves
        # (partition split) on sync+scalar, w4t2+v3[0:2] on gpsimd.
        # The scalar ring processes its half while the ACT engine loads
        # the activation table (act_warm), costing ACT nothing.
        xin_tile = const.tile([H, L + 2 * MB], BF16)
        x1t_bf = xin_tile[:, 0:L]
        negx2_f = xin_tile[:, L:].bitcast(F32)
        wv3_tile = const.tile([H, C + MB * C], BF16)
        w4t2_bf = wv3_tile[:, 0:C]
        v3_bf = wv3_tile[:, C:]
        ones_bf = const.tile([1, LCHUNK], BF16)

        nc.scalar.dma_start(xin_tile[64:128, :], xin[64:128, :])
        nc.sync.dma_start(xin_tile[0:64, :], xin[0:64, :])
        nc.gpsimd.dma_start(wv3_tile[:, 0:WVA], wv3[:, 0:WVA])
        nc.vector.memset(ones_bf[:], 1.0)
        # preload the ACT activation table off the critical path
        act_warm = const.tile([1, LCHUNK], BF16)
        nc.scalar.activation(act_warm[:], ones_bf[:],
                             mybir.ActivationFunctionType.Relu)

        # ---- main loop over m-blocks ----
        # Drains emitted one block late so they queue behind the next
        # block's D work in ACT's queue, prioritizing D production.
        pend = None

        def emit_drain(p):
            # psa3_/psb3_ are SEPARATE psum tiles: cross-engine readers of
            # one tile serialize in the framework, so the final block's
            # DVE+ACT halves only run in parallel with two tiles.
            ms_, psa3_, psb3_, last = p
            o_sb = opool.tile([LCHUNK, N_LC * CHUNK_F], BF16)
            o3 = o_sb[:].rearrange("p (lc c) -> p lc c", c=CHUNK_F)
            if not last:
                nc.scalar.copy(o3[:, 0:2], psa3_)
                nc.scalar.copy(o3[:, 2:4], psb3_)
                nc.sync.dma_start(out[ms_], o_sb[:])
            else:  # split engines + 2 DMAs on 2 queues for a short tail
                nc.vector.tensor_copy(o3[:, 0:2], psa3_)
                nc.sync.dma_start(out[ms_, :, 0 : 2 * CHUNK_F],
                                  o_sb[:, 0 : 2 * CHUNK_F])
                nc.scalar.copy(o3[:, 2:4], psb3_)
                nc.scalar.dma_start(out[ms_, :, 2 * CHUNK_F :],
                                    o_sb[:, 2 * CHUNK_F :])

        for ms in range(N_MS):
            # D tiles for this block
            dts = []
            pat = D_PATS[D_NV[ms]]
            for j in range(MSUB):
                m = ms * MSUB + j
                dt_ = dpool.tile([H, L], BF16, tag="d")
                if pat[j] == "V":
                    nc.vector.tensor_scalar(
                        dt_[:], x1t_bf, negx2_f[:, m : m + 1], 0.0,
                        op0=mybir.AluOpType.add, op1=mybir.AluOpType.max)
                else:
                    nc.scalar.activation(
                        dt_[:], x1t_bf, mybir.ActivationFunctionType.Relu,
                        bias=negx2_f[:, m : m + 1], scale=1.0)
                dts.append(dt_)

            if ms == 0:
                # v3 bulk rides second on the gpsimd ring; lands ~15.5us,
                # needed by block 2's t3 (~17.5us)
                nc.gpsimd.dma_start(wv3_tile[:, WVA:], wv3[:, WVA:])
            if pend is not None:
                emit_drain(pend)

            ps_a = psum.tile([LCHUNK, 2 * PS_STRIDE], F32, tag="ps")
            ps_b = psum.tile([LCHUNK, 2 * PS_STRIDE], F32, tag="ps")
            pss = [ps_a, ps_a, ps_b, ps_b]
            # t3 (+t1 fold) opens each group full-width (PSUM zeroing is
            # bank-granular), then the t4s accumulate j-major.
            for lc in range(N_LC):
                nc.tensor.matmul(
                    pss[lc][:, (lc % 2) * PS_STRIDE :
                            (lc % 2) * PS_STRIDE + CHUNK_F],
                    x1t_bf[:, lc * LCHUNK : (lc + 1) * LCHUNK],
                    v3_bf[:, ms * CHUNK_F : (ms + 1) * CHUNK_F],
                    start=True, stop=False, skip_group_check=True)
            for j in range(MSUB):
                for lc in range(N_LC):
                    base = (lc % 2) * PS_STRIDE
                    nc.tensor.matmul(
                        pss[lc][:, base + j * C : base + (j + 1) * C],
                        dts[j][:, lc * LCHUNK : (lc + 1) * LCHUNK],
                        w4t2_bf[:],
                        start=False, stop=(j == MSUB - 1),
                        skip_group_check=True)

            psa3 = ps_a[:].rearrange("p (lc x) -> p lc x",
                                     x=PS_STRIDE)[:, :, 0:CHUNK_F]
            psb3 = ps_b[:].rearrange("p (lc x) -> p lc x",
                                     x=PS_STRIDE)[:, :, 0:CHUNK_F]
            pend = (ms, psa3, psb3, ms == N_MS - 1)
        emit_drain(pend)
    return nc


_COMPILED = {}


def _get_compiled():
    if "nc" not in _COMPILED:
        nc = bacc.Bacc("TRN2", target_bir_lowering=False, debug=False,
                       num_devices=N_CORES)
        build_kernel(nc)
        nc.compile()
        _COMPILED["nc"] = nc
    return _COMPILED["nc"]


def make_in_maps(x1, x2, W, b):
    W1, W2, W3, W4 = (W[:, 0:H], W[:, H : 2 * H], W[:, 2 * H : 3 * H],
                      W[:, 3 * H : 4 * H])
    w13 = (W1 - W4).T.astype(np.float32)          # [H, C]
    w3t = W3.T.astype(np.float32)                 # [H, C]
    w4t2 = (2.0 * W4).T.astype(np.float32)        # [H, C]
    in_maps = []
    for cid in range(N_CORES):
        bb, mblk = cid // 4, cid % 4
        m0 = mblk * MB
        x2blk = x2[bb, m0 : m0 + MB]              # [MB, H]
        x2t = x2blk.T                             # [H, MB]
        # V3[h, m*C+c] = x2t[h,m]*W3T[h,c] + (W1-W4)T[h,c]
        v3 = x2t[:, :, None] * w3t[:, None, :] + w13[:, None, :]
        wv3 = np.concatenate([w4t2, v3.reshape(H, MB * C)], axis=1)
        # xin: x1t bf16 cols 0:512 | negx2 f32 bytes viewed as bf16 cols
        # 512:768 (bitcast back to f32 on device)
        xin = np.empty((H, L + 2 * MB), dtype=BF16_NP)
        xin[:, 0:L] = x1[bb].T.astype(BF16_NP)
        negx2 = np.ascontiguousarray(-x2t.astype(np.float32))
        xin[:, L:] = negx2.view(np.uint16).view(BF16_NP).reshape(H, 2 * MB)
        in_maps.append({
            "xin": xin,
            "wv3": np.ascontiguousarray(wv3.astype(BF16_NP)),
        })
    return in_maps


def t2_bias(x2, W, b):
    """Host-side t2 term: x2 @ (W2+W4).T + bias, [B, L, C] f32."""
    W2 = W[:, H : 2 * H]
    W4 = W[:, 3 * H : 4 * H]
    return (x2 @ (W2 + W4).T + b).astype(np.float32)


def run_on_device(x1, x2, W, b, trace=False, trace_kwargs=None):
    nc = _get_compiled()
    in_maps = make_in_maps(x1, x2, W, b)
    res = bass_utils.run_bass_kernel_spmd(
        nc, in_maps, core_ids=list(range(N_CORES)), trace=trace,
        **(trace_kwargs or {}))
    t2 = t2_bias(x2, W, b)                        # [B, L, C]
    full = np.empty((B, L, L, C), dtype=np.float32)
    for cid in range(N_CORES):
        bb, mblk = cid // 4, cid % 4
        m0 = mblk * MB
        # device out is [ms, p, (lc, j, c)]; l = lc*128+p, m = ms*16+j
        dev = (np.asarray(res.results[cid]["out"])
               .reshape(N_MS, LCHUNK, N_LC, MSUB, C)
               .transpose(2, 1, 0, 3, 4)
               .reshape(L, MB, C).astype(np.float32))
        full[bb, :, m0 : m0 + MB, :] = (
            dev + t2[bb, m0 : m0 + MB, :][None, :, :])
    return full, res


def kernel(x1, x2, W, b):
    x1 = np.asarray(x1, dtype=np.float32)
    x2 = np.asarray(x2, dtype=np.float32)
    W = np.asarray(W, dtype=np.float32)
    b = np.asarray(b, dtype=np.float32)
    full, _ = run_on_device(x1, x2, W, b, trace=False)
    return full


# revision 27
# speedup vs baseline: 1.0481x; 1.0481x over previous
"""Trainium2 Bass kernel for the BiaffineLayer problem.

Math (per batch b):
  out[l, m, c] = x1[l] @ W1[c] + x2[m] @ W2[c]
              + sum_h x1[l,h] * x2[m,h] * W3[c,h]
              + sum_h |x1[l,h] - x2[m,h]| * W4[c,h] + bias[c]
  shapes: x1, x2 [2, 512, 128]; W [25, 512]; bias [25]; out [2, 512, 512, 25]

Sharding: 8 cores = 2 batches x 4 m-blocks of 128 columns. Core (b, mb) gets
full x1[b] and its x2[b, m0:m0+128] block; it produces out[b, :, m0:m0+MB, :].

Decomposition, with |d| = 2*relu(d) - d and d = x1 - x2[m]:
  out = x1t' @ V3  +  D_m' @ (2 W4T)  +  T2B
where (host-precomputed except D):
  V3[h,(m,c)] = x2[m,h]*W3[c,h] + (W1-W4)[c,h]   (t3 + t1 - t4's -x1*W4 part)
  T2B[m,c]    = x2[m] @ (W2+W4)T + b             (added on the HOST during
                                                  unshard - pure per-(m,c))
  D_m[h,l]    = relu(x1[l,h] - x2[m,h])          (device, pairwise)

Measured HW rates (microbenched this session):
  - DVE tensor_scalar = ~129ns fixed + 0.26ns/elem (263ns per [128,512]
    D tile; the bf16 2x mode is already engaged; the f32 ptr scalar costs
    a fixed ~60ns; no 4x mode reachable). D-gen on DVE is THE critical
    resource: 108 tiles x 263ns = 28.4us.
  - ACT relu-with-bias ~710ns/tile in-kernel, ACT [128,800] PSUM drain
    ~930ns. ACT carries 20 D tiles + the drains and still has slack.
  - PE: t4 LDW+MM pair 25ns, t3 400-col MM 168ns; PE total ~18us, never
    critical. fp8 gains nothing (LDW is row-bound, DVE/ACT fp8-out is
    1.5x SLOWER, and e4m3 would not fit the error budget anyway).
  - DMA: ~0.7us issue + ~1.5us ring startup + ~17ns per partition row;
    only sync/scalar/gpsimd queues can issue; a ring's second DMA starts
    ~1us after its first; large out-DMAs on the gpsimd ring can hit a
    ~19us descriptor pathology (keep them on sync).
  - Framework overhead: ~7.3us prologue (engine barriers etc.) + ~3.4us
    counted teardown; both effectively fixed.

Schedule:
  - xin = x1t|negx2-bytes merged into one [H, 768] bf16 tensor so negx2
    rides x1t's fat 1536B rows (a separate 512B-row DMA measured ~5x
    slower and landed last); split by PARTITION halves (descriptor count
    scales with rows) on the sync+scalar rings, first-on-ring. w4t2+v3
    merged into wv3; its first chunk is first on the gpsimd ring, the
    bulk rides second there (lands ~15.5us, needed ~17.5us).
  - Per 16-m block: DVE makes 12-14 D tiles, ACT 2-4 (placed EARLY in j
    so the block's end never waits on the slower ACT; cadence is then
    pure DVE). Two 2-bank PSUM tiles per block (pool depth 2 blocks);
    t3 opens each group full-width, t4s accumulate j-major.
  - Drains (2 ACT copies + 1 sync out-DMA per block) emitted one block
    late so they run in ACT's idle tail. The final block is DVE-light
    (nV=12) and its drain splits DVE+ACT in PARALLEL - this requires the
    two halves to read SEPARATE psum tiles (cross-engine readers of one
    tile serialize in the framework) - with its two out-DMAs on the
    sync+scalar queues.
  - Output DRAM layout is block-major [ms, p, lc*c]: each partition's
    3200B is one contiguous DMA descriptor (vs 512x800B strided rows
    for an l-major layout), shortening the tail transfers.
  - Host converts bf16 output to f32, adds T2B, untangles the
    block-major layout, and reassembles [B, L, L, C].

Measured end-to-end (fresh device): ~45.2-45.6us. Anatomy: 7.3us fixed
prologue; D-gen starts ~10.0us (DMA ring startup+transfer); 28.45us
gapless DVE-bound region; PE stop +0.35; parallel drain halves +1.0;
final DMA issue+transfer+latency ~3.1; fixed barrier epilogue ~2.5.
"""

import sys

sys.path.insert(0, "/opt/trn_rl_repo")

from contextlib import ExitStack

import ml_dtypes
import numpy as np

import concourse.bass as bass
import concourse.tile as tile
from concourse import bacc, bass_utils, mybir

F32 = mybir.dt.float32
BF16 = mybir.dt.bfloat16
BF16_NP = ml_dtypes.bfloat16

B, L, H, C = 2, 512, 128, 25
MB = 128            # m-block per core
N_CORES = 8
MSUB = 16           # m's per psum block
N_MS = MB // MSUB   # 8 blocks over the m-block
LCHUNK = 128
N_LC = L // LCHUNK  # 4 l-chunks
CHUNK_F = MSUB * C  # 400 psum free columns per l-chunk slice
PS_STRIDE = 512     # psum bank stride (f32 elems) per l-chunk slice

# Per-block D-tile engine split: V=DVE, A=ACT. sum V = 108.
# ACT's per-block queue is [A-tiles..., drain(k-1)]; A positions are EARLY
# (j=1,3,..) so the block's end never waits on the slower ACT engine --
# the cadence is then set purely by DVE (nV x 263ns). The drain runs in
# ACT's idle tail of each block. First and last blocks are ACT-heavy:
# block 0 has no drain yet, and a DVE-light final block shortens the tail
# (DVE finishes early and takes half of the final drain).
D_PATS = {
    12: "VAVAVAVAVVVVVVVV",   # V=12 A=4 at j=1,3,5,7
    13: "VAVAVAVVVVVVVVVV",   # V=13 A=3 at j=1,3,5
    14: "VAVAVVVVVVVVVVVV",   # V=14 A=2 at j=1,3
}
D_NV = [12, 14, 13, 14, 13, 14, 14, 12]   # sum = 106


def build_kernel(nc: bass.Bass, repeat: int = 1):
    # xin = x1t bf16 [H, 512] cols 0:512 | negx2 f32 [H, 128] as bf16-viewed
    # bytes in cols 512:768. Merging rides negx2's bytes on x1t's fat
    # 1536B rows: one DMA per partition-half at full bandwidth instead of
    # a separate 512B-row DMA that measured ~5x slower and landed last.
    xin = nc.dram_tensor("xin", (H, L + 2 * MB), BF16, kind="ExternalInput").ap()
    # w4t2 (25 cols) | v3 (3200 cols) merged
    wv3 = nc.dram_tensor("wv3", (H, C + MB * C), BF16, kind="ExternalInput").ap()
    # block-major output [ms, p, lc*c]: each partition's 3200B is one
    # contiguous DMA descriptor (vs 512x800B for an l-major layout); the
    # host untangles the layout during unshard
    out = nc.dram_tensor("out", (N_MS, LCHUNK, N_LC * CHUNK_F), BF16,
                         kind="ExternalOutput").ap()

    WVA = C + 2 * CHUNK_F        # w4t2 + v3 blocks 0-1

    with tile.TileContext(nc) as tc, ExitStack() as ctx:
      const = ctx.enter_context(tc.tile_pool(name="const", bufs=1))
      dpool = ctx.enter_context(tc.tile_pool(name="dpool", bufs=128))
      opool = ctx.enter_context(tc.tile_pool(name="opool", bufs=10))
      psum = ctx.enter_context(tc.tile_pool(name="psum", bufs=4, space="PSUM"))
      for _rep in range(repeat):
        # ---- input loads ----
        # DMA queues are sync/scalar/gpsimd only. A DMA costs ~1.3us
        # fixed startup + transfer at ~230GB/s for >=1KB rows, and a
        # ring's SECOND DMA starts ~1us after its first finishes. So
        # everything D-gen needs rides FIRST on a ring: xin halves
        # (partition split) on sync+scalar, w4t2+v3[0:2] on gpsimd.
        # The scalar ring processes its half while the ACT engine loads
        # the activation table (act_warm), costing ACT nothing.
        xin_tile = const.tile([H, L + 2 * MB], BF16)
        x1t_bf = xin_tile[:, 0:L]
        negx2_f = xin_tile[:, L:].bitcast(F32)
        wv3_tile = const.tile([H, C + MB * C], BF16)
        w4t2_bf = wv3_tile[:, 0:C]
        v3_bf = wv3_tile[:, C:]
        ones_bf = const.tile([1, LCHUNK], BF16)

        nc.scalar.dma_start(xin_tile[64:128, :], xin[64:128, :])
        nc.sync.dma_start(xin_tile[0:64, :], xin[0:64, :])
        nc.gpsimd.dma_start(wv3_tile[:, 0:WVA], wv3[:, 0:WVA])
        nc.vector.memset(ones_bf[:], 1.0)
        # preload the ACT activation table off the critical path
        act_warm = const.tile([1, LCHUNK], BF16)
        nc.scalar.activation(act_warm[:], ones_bf[:],
                             mybir.ActivationFunctionType.Relu)

        # ---- main loop over m-blocks ----
        # Drains emitted one block late so they queue behind the next
        # block's D work in ACT's queue, prioritizing D production.
        pend = None

        def emit_drain(p):
            # psa3_/psb3_ are SEPARATE psum tiles: cross-engine readers of
            # one tile serialize in the framework, so the final block's
            # DVE+ACT halves only run in parallel with two tiles.
            ms_, psa3_, psb3_, last = p
            o_sb = opool.tile([LCHUNK, N_LC * CHUNK_F], BF16)
            o3 = o_sb[:].rearrange("p (lc c) -> p lc c", c=CHUNK_F)
            if not last:
                nc.scalar.copy(o3[:, 0:2], psa3_)
                nc.scalar.copy(o3[:, 2:4], psb3_)
                nc.sync.dma_start(out[ms_], o_sb[:])
            else:  # split engines + 2 DMAs on 2 queues for a short tail
                nc.vector.tensor_copy(o3[:, 0:2], psa3_)
                nc.sync.dma_start(out[ms_, :, 0 : 2 * CHUNK_F],
                                  o_sb[:, 0 : 2 * CHUNK_F])
                nc.scalar.copy(o3[:, 2:4], psb3_)
                nc.scalar.dma_start(out[ms_, :, 2 * CHUNK_F :],
                                    o_sb[:, 2 * CHUNK_F :])

        for ms in range(N_MS):
            # D tiles for this block
            dts = []
            pat = D_PATS[D_NV[ms]]
            for j in range(MSUB):
                m = ms * MSUB + j
                dt_ = dpool.tile([H, L], BF16, tag="d")
                if pat[j] == "V":
                    nc.vector.tensor_scalar(
                        dt_[:], x1t_bf, negx2_f[:, m : m + 1], 0.0,
                        op0=mybir.AluOpType.add, op1=mybir.AluOpType.max)
                else:
                    nc.scalar.activation(
                        dt_[:], x1t_bf, mybir.ActivationFunctionType.Relu,
                        bias=negx2_f[:, m : m + 1], scale=1.0)
                dts.append(dt_)

            if ms == 0:
                # v3 bulk rides second on the gpsimd ring; lands ~15.5us,
                # needed by block 2's t3 (~17.5us)
                nc.gpsimd.dma_start(wv3_tile[:, WVA:], wv3[:, WVA:])
            if pend is not None:
                emit_drain(pend)

            ps_a = psum.tile([LCHUNK, 2 * PS_STRIDE], F32, tag="ps")
            ps_b = psum.tile([LCHUNK, 2 * PS_STRIDE], F32, tag="ps")
            pss = [ps_a, ps_a, ps_b, ps_b]
            # t3 (+t1 fold) opens each group full-width (PSUM zeroing is
            # bank-granular), then the t4s accumulate j-major.
            for lc in range(N_LC):
                nc.tensor.matmul(
                    pss[lc][:, (lc % 2) * PS_STRIDE :
                            (lc % 2) * PS_STRIDE + CHUNK_F],
                    x1t_bf[:, lc * LCHUNK : (lc + 1) * LCHUNK],
                    v3_bf[:, ms * CHUNK_F : (ms + 1) * CHUNK_F],
                    start=True, stop=False, skip_group_check=True)
            for j in range(MSUB):
                for lc in range(N_LC):
                    base = (lc % 2) * PS_STRIDE
                    nc.tensor.matmul(
                        pss[lc][:, base + j * C : base + (j + 1) * C],
                        dts[j][:, lc * LCHUNK : (lc + 1) * LCHUNK],
                        w4t2_bf[:],
                        start=False, stop=(j == MSUB - 1),
                        skip_group_check=True)

            psa3 = ps_a[:].rearrange("p (lc x) -> p lc x",
                                     x=PS_STRIDE)[:, :, 0:CHUNK_F]
            psb3 = ps_b[:].rearrange("p (lc x) -> p lc x",
                                     x=PS_STRIDE)[:, :, 0:CHUNK_F]
            pend = (ms, psa3, psb3, ms == N_MS - 1)
        emit_drain(pend)
    return nc


_COMPILED = {}


def _get_compiled():
    if "nc" not in _COMPILED:
        nc = bacc.Bacc("TRN2", target_bir_lowering=False, debug=False,
                       num_devices=N_CORES)
        build_kernel(nc)
        nc.compile()
        _COMPILED["nc"] = nc
    return _COMPILED["nc"]


def make_in_maps(x1, x2, W, b):
    W1, W2, W3, W4 = (W[:, 0:H], W[:, H : 2 * H], W[:, 2 * H : 3 * H],
                      W[:, 3 * H : 4 * H])
    w13 = (W1 - W4).T.astype(np.float32)          # [H, C]
    w3t = W3.T.astype(np.float32)                 # [H, C]
    w4t2 = (2.0 * W4).T.astype(np.float32)        # [H, C]
    in_maps = []
    for cid in range(N_CORES):
        bb, mblk = cid // 4, cid % 4
        m0 = mblk * MB
        x2blk = x2[bb, m0 : m0 + MB]              # [MB, H]
        x2t = x2blk.T                             # [H, MB]
        # V3[h, m*C+c] = x2t[h,m]*W3T[h,c] + (W1-W4)T[h,c]
        v3 = x2t[:, :, None] * w3t[:, None, :] + w13[:, None, :]
        wv3 = np.concatenate([w4t2, v3.reshape(H, MB * C)], axis=1)
        # xin: x1t bf16 cols 0:512 | negx2 f32 bytes viewed as bf16 cols
        # 512:768 (bitcast back to f32 on device)
        xin = np.empty((H, L + 2 * MB), dtype=BF16_NP)
        xin[:, 0:L] = x1[bb].T.astype(BF16_NP)
        negx2 = np.ascontiguousarray(-x2t.astype(np.float32))
        xin[:, L:] = negx2.view(np.uint16).view(BF16_NP).reshape(H, 2 * MB)
        in_maps.append({
            "xin": xin,
            "wv3": np.ascontiguousarray(wv3.astype(BF16_NP)),
        })
    return in_maps


def t2_bias(x2, W, b):
    """Host-side t2 term: x2 @ (W2+W4).T + bias, [B, L, C] f32."""
    W2 = W[:, H : 2 * H]
    W4 = W[:, 3 * H : 4 * H]
    return (x2 @ (W2 + W4).T + b).astype(np.float32)


def run_on_device(x1, x2, W, b, trace=False, trace_kwargs=None):
    nc = _get_compiled()
    in_maps = make_in_maps(x1, x2, W, b)
    res = bass_utils.run_bass_kernel_spmd(
        nc, in_maps, core_ids=list(range(N_CORES)), trace=trace,
        **(trace_kwargs or {}))
    t2 = t2_bias(x2, W, b)                        # [B, L, C]
    full = np.empty((B, L, L, C), dtype=np.float32)
    for cid in range(N_CORES):
        bb, mblk = cid // 4, cid % 4
        m0 = mblk * MB
        # device out is [ms, p, (lc, j, c)]; l = lc*128+p, m = ms*16+j
        dev = (np.asarray(res.results[cid]["out"])
               .reshape(N_MS, LCHUNK, N_LC, MSUB, C)
               .transpose(2, 1, 0, 3, 4)
               .reshape(L, MB, C).astype(np.float32))
        full[bb, :, m0 : m0 + MB, :] = (
            dev + t2[bb, m0 : m0 + MB, :][None, :, :])
    return full, res


def kernel(x1, x2, W, b):
    x1 = np.asarray(x1, dtype=np.float32)
    x2 = np.asarray(x2, dtype=np.float32)
    W = np.asarray(W, dtype=np.float32)
    b = np.asarray(b, dtype=np.float32)
    full, _ = run_on_device(x1, x2, W, b, trace=False)
    return full


# revision 28
# speedup vs baseline: 1.0502x; 1.0020x over previous
"""Trainium2 Bass kernel for the BiaffineLayer problem.

Math (per batch b):
  out[l, m, c] = x1[l] @ W1[c] + x2[m] @ W2[c]
              + sum_h x1[l,h] * x2[m,h] * W3[c,h]
              + sum_h |x1[l,h] - x2[m,h]| * W4[c,h] + bias[c]
  shapes: x1, x2 [2, 512, 128]; W [25, 512]; bias [25]; out [2, 512, 512, 25]

Sharding: 8 cores = 2 batches x 4 m-blocks of 128 columns. Core (b, mb) gets
full x1[b] and its x2[b, m0:m0+128] block; it produces out[b, :, m0:m0+MB, :].

Decomposition, with |d| = 2*relu(d) - d and d = x1 - x2[m]:
  out = x1t' @ V3  +  D_m' @ (2 W4T)  +  T2B
where (host-precomputed except D):
  V3[h,(m,c)] = x2[m,h]*W3[c,h] + (W1-W4)[c,h]   (t3 + t1 - t4's -x1*W4 part)
  T2B[m,c]    = x2[m] @ (W2+W4)T + b             (added on the HOST during
                                                  unshard - pure per-(m,c))
  D_m[h,l]    = relu(x1[l,h] - x2[m,h])          (device, pairwise)

Measured HW rates (microbenched this session):
  - DVE tensor_scalar = ~129ns fixed + 0.26ns/elem (263ns per [128,512]
    D tile; the bf16 2x mode is already engaged; the f32 ptr scalar costs
    a fixed ~60ns; no 4x mode reachable). D-gen on DVE is THE critical
    resource: 106 tiles x 263ns = 27.9us.
  - ACT relu-with-bias ~710ns/tile in-kernel, ACT [128,800] PSUM drain
    ~930ns. ACT carries 20 D tiles + the drains and still has slack.
  - PE: t4 LDW+MM pair 25ns, t3 400-col MM 168ns; PE total ~18us, never
    critical. fp8 gains nothing (LDW is row-bound, DVE/ACT fp8-out is
    1.5x SLOWER, and e4m3 would not fit the error budget anyway).
  - DMA: ~0.7us issue + ~1.5us ring startup + ~17ns per partition row;
    only sync/scalar/gpsimd queues can issue; a ring's second DMA starts
    ~1us after its first; large out-DMAs on the gpsimd ring can hit a
    ~19us descriptor pathology (keep them on sync).
  - Framework overhead: ~7.3us prologue (engine barriers etc.) + ~3.4us
    counted teardown; both effectively fixed.

Schedule:
  - xin = x1t|negx2-bytes merged into one [H, 768] bf16 tensor so negx2
    rides x1t's fat 1536B rows (a separate 512B-row DMA measured ~5x
    slower and landed last); split by PARTITION halves (descriptor count
    scales with rows) on the sync+scalar rings, first-on-ring. w4t2+v3
    merged into wv3; its first chunk is first on the gpsimd ring, the
    bulk rides second there (lands ~15.5us, needed ~17.5us).
  - Per 16-m block: DVE makes 12-14 D tiles, ACT 2-4 (placed EARLY in j
    so the block's end never waits on the slower ACT; cadence is then
    pure DVE). Two 2-bank PSUM tiles per block (pool depth 2 blocks);
    t3 opens each group full-width, t4s accumulate j-major.
  - Drains (2 ACT copies + 1 sync out-DMA per block) emitted one block
    late so they run in ACT's idle tail. The final block is DVE-light
    (nV=12) and its drain splits DVE+ACT in PARALLEL - this requires the
    two halves to read SEPARATE psum tiles (cross-engine readers of one
    tile serialize in the framework) - with its two out-DMAs on the
    sync+scalar queues.
  - Output DRAM layout is block-major [ms, p, lc*c]: each partition's
    3200B is one contiguous DMA descriptor (vs 512x800B strided rows
    for an l-major layout), shortening the tail transfers.
  - Host converts bf16 output to f32, adds T2B, untangles the
    block-major layout, and reassembles [B, L, L, C].

Measured end-to-end (fresh device): ~45.2-45.6us. Anatomy: 7.3us fixed
prologue; D-gen starts ~10.0us (DMA ring startup+transfer); 28.45us
gapless DVE-bound region; PE stop +0.35; parallel drain halves +1.0;
final DMA issue+transfer+latency ~3.1; fixed barrier epilogue ~2.5.
"""

import sys

sys.path.insert(0, "/opt/trn_rl_repo")

from contextlib import ExitStack

import ml_dtypes
import numpy as np

import concourse.bass as bass
import concourse.tile as tile
from concourse import bacc, bass_utils, mybir

F32 = mybir.dt.float32
BF16 = mybir.dt.bfloat16
BF16_NP = ml_dtypes.bfloat16

B, L, H, C = 2, 512, 128, 25
MB = 128            # m-block per core
N_CORES = 8
MSUB = 16           # m's per psum block
N_MS = MB // MSUB   # 8 blocks over the m-block
LCHUNK = 128
N_LC = L // LCHUNK  # 4 l-chunks
CHUNK_F = MSUB * C  # 400 psum free columns per l-chunk slice
PS_STRIDE = 512     # psum bank stride (f32 elems) per l-chunk slice

# Per-block D-tile engine split: V=DVE, A=ACT. sum V = 106 (totals
# balance: DVE 106x263=27.9us vs ACT 22 tiles + drains ~27.7us; drains
# may slip a block late within the 2-block PSUM depth, absorbed by the
# PE's ~1.4us/block slack, so ACT's TOTAL is the constraint, not its
# per-block share).
# ACT's per-block queue is [A-tiles..., drain(k-1)]; A positions are EARLY
# (j=1,3,..) so the block's end never waits on the slower ACT engine --
# the cadence is then set purely by DVE (nV x 263ns). The drain runs in
# ACT's idle tail of each block. First and last blocks are ACT-heavy:
# block 0 has no drain yet, and a DVE-light final block shortens the tail
# (DVE finishes early and takes half of the final drain).
D_PATS = {
    12: "VAVAVAVAVVVVVVVV",   # V=12 A=4 at j=1,3,5,7
    13: "VAVAVAVVVVVVVVVV",   # V=13 A=3 at j=1,3,5
    14: "VAVAVVVVVVVVVVVV",   # V=14 A=2 at j=1,3
}
D_NV = [12, 14, 13, 14, 13, 14, 14, 12]   # sum = 106


def build_kernel(nc: bass.Bass, repeat: int = 1):
    # xin = x1t bf16 [H, 512] cols 0:512 | negx2 f32 [H, 128] as bf16-viewed
    # bytes in cols 512:768. Merging rides negx2's bytes on x1t's fat
    # 1536B rows: one DMA per partition-half at full bandwidth instead of
    # a separate 512B-row DMA that measured ~5x slower and landed last.
    xin = nc.dram_tensor("xin", (H, L + 2 * MB), BF16, kind="ExternalInput").ap()
    # w4t2 (25 cols) | v3 (3200 cols) merged
    wv3 = nc.dram_tensor("wv3", (H, C + MB * C), BF16, kind="ExternalInput").ap()
    # block-major output [ms, p, lc*c]: each partition's 3200B is one
    # contiguous DMA descriptor (vs 512x800B for an l-major layout); the
    # host untangles the layout during unshard
    out = nc.dram_tensor("out", (N_MS, LCHUNK, N_LC * CHUNK_F), BF16,
                         kind="ExternalOutput").ap()

    WVA = C + 2 * CHUNK_F        # w4t2 + v3 blocks 0-1

    with tile.TileContext(nc) as tc, ExitStack() as ctx:
      const = ctx.enter_context(tc.tile_pool(name="const", bufs=1))
      dpool = ctx.enter_context(tc.tile_pool(name="dpool", bufs=128))
      opool = ctx.enter_context(tc.tile_pool(name="opool", bufs=10))
      psum = ctx.enter_context(tc.tile_pool(name="psum", bufs=4, space="PSUM"))
      for _rep in range(repeat):
        # ---- input loads ----
        # DMA queues are sync/scalar/gpsimd only. A DMA costs ~1.3us
        # fixed startup + transfer at ~230GB/s for >=1KB rows, and a
        # ring's SECOND DMA starts ~1us after its first finishes. So
        # everything D-gen needs rides FIRST on a ring: xin halves
        # (partition split) on sync+scalar, w4t2+v3[0:2] on gpsimd.
        # The scalar ring processes its half while the ACT engine loads
        # the activation table (act_warm), costing ACT nothing.
        xin_tile = const.tile([H, L + 2 * MB], BF16)
        x1t_bf = xin_tile[:, 0:L]
        negx2_f = xin_tile[:, L:].bitcast(F32)
        wv3_tile = const.tile([H, C + MB * C], BF16)
        w4t2_bf = wv3_tile[:, 0:C]
        v3_bf = wv3_tile[:, C:]
        ones_bf = const.tile([1, LCHUNK], BF16)

        nc.scalar.dma_start(xin_tile[64:128, :], xin[64:128, :])
        nc.sync.dma_start(xin_tile[0:64, :], xin[0:64, :])
        nc.gpsimd.dma_start(wv3_tile[:, 0:WVA], wv3[:, 0:WVA])
        nc.vector.memset(ones_bf[:], 1.0)
        # preload the ACT activation table off the critical path
        act_warm = const.tile([1, LCHUNK], BF16)
        nc.scalar.activation(act_warm[:], ones_bf[:],
                             mybir.ActivationFunctionType.Relu)

        # ---- main loop over m-blocks ----
        # Drains emitted one block late so they queue behind the next
        # block's D work in ACT's queue, prioritizing D production.
        pend = None

        def emit_drain(p):
            # psa3_/psb3_ are SEPARATE psum tiles: cross-engine readers of
            # one tile serialize in the framework, so the final block's
            # DVE+ACT halves only run in parallel with two tiles.
            ms_, psa3_, psb3_, last = p
            o_sb = opool.tile([LCHUNK, N_LC * CHUNK_F], BF16)
            o3 = o_sb[:].rearrange("p (lc c) -> p lc c", c=CHUNK_F)
            if not last:
                nc.scalar.copy(o3[:, 0:2], psa3_)
                nc.scalar.copy(o3[:, 2:4], psb3_)
                nc.sync.dma_start(out[ms_], o_sb[:])
            else:  # split engines + 2 DMAs on 2 queues for a short tail
                nc.vector.tensor_copy(o3[:, 0:2], psa3_)
                nc.sync.dma_start(out[ms_, :, 0 : 2 * CHUNK_F],
                                  o_sb[:, 0 : 2 * CHUNK_F])
                nc.scalar.copy(o3[:, 2:4], psb3_)
                nc.scalar.dma_start(out[ms_, :, 2 * CHUNK_F :],
                                    o_sb[:, 2 * CHUNK_F :])

        for ms in range(N_MS):
            # D tiles for this block
            dts = []
            pat = D_PATS[D_NV[ms]]
            for j in range(MSUB):
                m = ms * MSUB + j
                dt_ = dpool.tile([H, L], BF16, tag="d")
                if pat[j] == "V":
                    nc.vector.tensor_scalar(
                        dt_[:], x1t_bf, negx2_f[:, m : m + 1], 0.0,
                        op0=mybir.AluOpType.add, op1=mybir.AluOpType.max)
                else:
                    nc.scalar.activation(
                        dt_[:], x1t_bf, mybir.ActivationFunctionType.Relu,
                        bias=negx2_f[:, m : m + 1], scale=1.0)
                dts.append(dt_)

            if ms == 0:
                # v3 bulk rides second on the gpsimd ring; lands ~15.5us,
                # needed by block 2's t3 (~17.5us)
                nc.gpsimd.dma_start(wv3_tile[:, WVA:], wv3[:, WVA:])
            if pend is not None:
                emit_drain(pend)

            ps_a = psum.tile([LCHUNK, 2 * PS_STRIDE], F32, tag="ps")
            ps_b = psum.tile([LCHUNK, 2 * PS_STRIDE], F32, tag="ps")
            pss = [ps_a, ps_a, ps_b, ps_b]
            # t3 (+t1 fold) opens each group full-width (PSUM zeroing is
            # bank-granular), then the t4s accumulate j-major.
            for lc in range(N_LC):
                nc.tensor.matmul(
                    pss[lc][:, (lc % 2) * PS_STRIDE :
                            (lc % 2) * PS_STRIDE + CHUNK_F],
                    x1t_bf[:, lc * LCHUNK : (lc + 1) * LCHUNK],
                    v3_bf[:, ms * CHUNK_F : (ms + 1) * CHUNK_F],
                    start=True, stop=False, skip_group_check=True)
            for j in range(MSUB):
                for lc in range(N_LC):
                    base = (lc % 2) * PS_STRIDE
                    nc.tensor.matmul(
                        pss[lc][:, base + j * C : base + (j + 1) * C],
                        dts[j][:, lc * LCHUNK : (lc + 1) * LCHUNK],
                        w4t2_bf[:],
                        start=False, stop=(j == MSUB - 1),
                        skip_group_check=True)

            psa3 = ps_a[:].rearrange("p (lc x) -> p lc x",
                                     x=PS_STRIDE)[:, :, 0:CHUNK_F]
            psb3 = ps_b[:].rearrange("p (lc x) -> p lc x",
                                     x=PS_STRIDE)[:, :, 0:CHUNK_F]
            pend = (ms, psa3, psb3, ms == N_MS - 1)
        emit_drain(pend)
    return nc


_COMPILED = {}


def _get_compiled():
    if "nc" not in _COMPILED:
        nc = bacc.Bacc("TRN2", target_bir_lowering=False, debug=False,
                       num_devices=N_CORES)
        build_kernel(nc)
        nc.compile()
        _COMPILED["nc"] = nc
    return _COMPILED["nc"]


def make_in_maps(x1, x2, W, b):
    W1, W2, W3, W4 = (W[:, 0:H], W[:, H : 2 * H], W[:, 2 * H : 3 * H],
                      W[:, 3 * H : 4 * H])
    w13 = (W1 - W4).T.astype(np.float32)          # [H, C]
    w3t = W3.T.astype(np.float32)                 # [H, C]
    w4t2 = (2.0 * W4).T.astype(np.float32)        # [H, C]
    in_maps = []
    for cid in range(N_CORES):
        bb, mblk = cid // 4, cid % 4
        m0 = mblk * MB
        x2blk = x2[bb, m0 : m0 + MB]              # [MB, H]
        x2t = x2blk.T                             # [H, MB]
        # V3[h, m*C+c] = x2t[h,m]*W3T[h,c] + (W1-W4)T[h,c]
        v3 = x2t[:, :, None] * w3t[:, None, :] + w13[:, None, :]
        wv3 = np.concatenate([w4t2, v3.reshape(H, MB * C)], axis=1)
        # xin: x1t bf16 cols 0:512 | negx2 f32 bytes viewed as bf16 cols
        # 512:768 (bitcast back to f32 on device)
        xin = np.empty((H, L + 2 * MB), dtype=BF16_NP)
        xin[:, 0:L] = x1[bb].T.astype(BF16_NP)
        negx2 = np.ascontiguousarray(-x2t.astype(np.float32))
        xin[:, L:] = negx2.view(np.uint16).view(BF16_NP).reshape(H, 2 * MB)
        in_maps.append({
            "xin": xin,
            "wv3": np.ascontiguousarray(wv3.astype(BF16_NP)),
        })
    return in_maps


def t2_bias(x2, W, b):
    """Host-side t2 term: x2 @ (W2+W4).T + bias, [B, L, C] f32."""
    W2 = W[:, H : 2 * H]
    W4 = W[:, 3 * H : 4 * H]
    return (x2 @ (W2 + W4).T + b).astype(np.float32)


def run_on_device(x1, x2, W, b, trace=False, trace_kwargs=None):
    nc = _get_compiled()
    in_maps = make_in_maps(x1, x2, W, b)
    res = bass_utils.run_bass_kernel_spmd(
        nc, in_maps, core_ids=list(range(N_CORES)), trace=trace,
        **(trace_kwargs or {}))
    t2 = t2_bias(x2, W, b)                        # [B, L, C]
    full = np.empty((B, L, L, C), dtype=np.float32)
    for cid in range(N_CORES):
        bb, mblk = cid // 4, cid % 4
        m0 = mblk * MB
        # device out is [ms, p, (lc, j, c)]; l = lc*128+p, m = ms*16+j
        dev = (np.asarray(res.results[cid]["out"])
               .reshape(N_MS, LCHUNK, N_LC, MSUB, C)
               .transpose(2, 1, 0, 3, 4)
               .reshape(L, MB, C).astype(np.float32))
        full[bb, :, m0 : m0 + MB, :] = (
            dev + t2[bb, m0 : m0 + MB, :][None, :, :])
    return full, res


def kernel(x1, x2, W, b):
    x1 = np.asarray(x1, dtype=np.float32)
    x2 = np.asarray(x2, dtype=np.float32)
    W = np.asarray(W, dtype=np.float32)
    b = np.asarray(b, dtype=np.float32)
    full, _ = run_on_device(x1, x2, W, b, trace=False)
    return full


# revision 29
# speedup vs baseline: 1.0579x; 1.0073x over previous
"""Trainium2 Bass kernel for the BiaffineLayer problem.

Math (per batch b):
  out[l, m, c] = x1[l] @ W1[c] + x2[m] @ W2[c]
              + sum_h x1[l,h] * x2[m,h] * W3[c,h]
              + sum_h |x1[l,h] - x2[m,h]| * W4[c,h] + bias[c]
  shapes: x1, x2 [2, 512, 128]; W [25, 512]; bias [25]; out [2, 512, 512, 25]

Sharding: 8 cores = 2 batches x 4 m-blocks of 128 columns. Core (b, mb) gets
full x1[b] and its x2[b, m0:m0+128] block; it produces out[b, :, m0:m0+MB, :].

Decomposition, with |d| = 2*relu(d) - d and d = x1 - x2[m]:
  out = x1t' @ V3  +  D_m' @ (2 W4T)  +  T2B
where (host-precomputed except D):
  V3[h,(m,c)] = x2[m,h]*W3[c,h] + (W1-W4)[c,h]   (t3 + t1 - t4's -x1*W4 part)
  T2B[m,c]    = x2[m] @ (W2+W4)T + b             (added on the HOST during
                                                  unshard - pure per-(m,c))
  D_m[h,l]    = relu(x1[l,h] - x2[m,h])          (device, pairwise)

Measured HW rates (microbenched this session):
  - DVE tensor_scalar = ~129ns fixed + 0.26ns/elem (263ns per [128,512]
    D tile; the bf16 2x mode is already engaged; the f32 ptr scalar costs
    a fixed ~60ns; no 4x mode reachable). D-gen on DVE is THE critical
    resource: 106 tiles x 263ns = 27.9us.
  - ACT relu-with-bias ~710ns/tile in-kernel, ACT [128,800] PSUM drain
    ~930ns. ACT carries 20 D tiles + the drains and still has slack.
  - PE: t4 LDW+MM pair 25ns, t3 400-col MM 168ns; PE total ~18us, never
    critical. fp8 gains nothing (LDW is row-bound, DVE/ACT fp8-out is
    1.5x SLOWER, and e4m3 would not fit the error budget anyway).
  - DMA: ~0.7us issue + ~1.5us ring startup + ~17ns per partition row;
    only sync/scalar/gpsimd queues can issue; a ring's second DMA starts
    ~1us after its first; large out-DMAs on the gpsimd ring can hit a
    ~19us descriptor pathology (keep them on sync).
  - Framework overhead: ~7.3us prologue (engine barriers etc.) + ~3.4us
    counted teardown; both effectively fixed.

Schedule:
  - xin = x1t|negx2-bytes merged into one [H, 768] bf16 tensor so negx2
    rides x1t's fat 1536B rows (a separate 512B-row DMA measured ~5x
    slower and landed last); split by PARTITION halves (descriptor count
    scales with rows) on the sync+scalar rings, first-on-ring. w4t2+v3
    merged into wv3; its first chunk is first on the gpsimd ring, the
    bulk rides second there (lands ~15.5us, needed ~17.5us).
  - Per 16-m block: DVE makes 12-14 D tiles, ACT 2-4 (placed EARLY in j
    so the block's end never waits on the slower ACT; cadence is then
    pure DVE). Two 2-bank PSUM tiles per block (pool depth 2 blocks);
    t3 opens each group full-width, t4s accumulate j-major.
  - Drains (2 ACT copies + 1 sync out-DMA per block) emitted one block
    late so they run in ACT's idle tail. The final block is DVE-light
    (nV=12) and its drain splits DVE+ACT in PARALLEL - this requires the
    two halves to read SEPARATE psum tiles (cross-engine readers of one
    tile serialize in the framework) - with its two out-DMAs on the
    sync+scalar queues.
  - Output DRAM layout is block-major [ms, p, lc*c]: each partition's
    3200B is one contiguous DMA descriptor (vs 512x800B strided rows
    for an l-major layout), shortening the tail transfers.
  - Host converts bf16 output to f32, adds T2B, untangles the
    block-major layout, and reassembles [B, L, L, C].

Measured end-to-end (fresh device): ~45.2-45.6us. Anatomy: 7.3us fixed
prologue; D-gen starts ~10.0us (DMA ring startup+transfer); 28.45us
gapless DVE-bound region; PE stop +0.35; parallel drain halves +1.0;
final DMA issue+transfer+latency ~3.1; fixed barrier epilogue ~2.5.
"""

import sys

sys.path.insert(0, "/opt/trn_rl_repo")

from contextlib import ExitStack

import ml_dtypes
import numpy as np

import concourse.bass as bass
import concourse.tile as tile
from concourse import bacc, bass_utils, mybir

F32 = mybir.dt.float32
BF16 = mybir.dt.bfloat16
BF16_NP = ml_dtypes.bfloat16

B, L, H, C = 2, 512, 128, 25
MB = 128            # m-block per core
N_CORES = 8
MSUB = 16           # m's per psum block
N_MS = MB // MSUB   # 8 blocks over the m-block
LCHUNK = 128
N_LC = L // LCHUNK  # 4 l-chunks
CHUNK_F = MSUB * C  # 400 psum free columns per l-chunk slice
PS_STRIDE = 512     # psum bank stride (f32 elems) per l-chunk slice

# Per-block D-tile engine split: V=DVE, A=ACT. sum V = 106 (totals
# balance: DVE 106x263=27.9us vs ACT 22 tiles + drains ~27.7us; drains
# may slip a block late within the 2-block PSUM depth, absorbed by the
# PE's ~1.4us/block slack, so ACT's TOTAL is the constraint, not its
# per-block share).
# ACT's per-block queue is [A-tiles..., drain(k-1)]; A positions are EARLY
# (j=1,3,..) so the block's end never waits on the slower ACT engine --
# the cadence is then set purely by DVE (nV x 263ns). The drain runs in
# ACT's idle tail of each block. First and last blocks are ACT-heavy:
# block 0 has no drain yet, and a DVE-light final block shortens the tail
# (DVE finishes early and takes half of the final drain).
D_PATS = {
    12: "VAVAVAVAVVVVVVVV",   # V=12 A=4 at j=1,3,5,7
    13: "VAVAVAVVVVVVVVVV",   # V=13 A=3 at j=1,3,5
    14: "VAVAVVVVVVVVVVVV",   # V=14 A=2 at j=1,3
}
D_NV = [12, 13, 13, 14, 13, 14, 14, 12]   # sum = 105


def build_kernel(nc: bass.Bass, repeat: int = 1):
    # xin = x1t bf16 [H, 512] cols 0:512 | negx2 f32 [H, 128] as bf16-viewed
    # bytes in cols 512:768. Merging rides negx2's bytes on x1t's fat
    # 1536B rows: one DMA per partition-half at full bandwidth instead of
    # a separate 512B-row DMA that measured ~5x slower and landed last.
    xin = nc.dram_tensor("xin", (H, L + 2 * MB), BF16, kind="ExternalInput").ap()
    # w4t2 (25 cols) | v3 (3200 cols) merged
    wv3 = nc.dram_tensor("wv3", (H, C + MB * C), BF16, kind="ExternalInput").ap()
    # block-major output [ms, p, lc*c]: each partition's 3200B is one
    # contiguous DMA descriptor (vs 512x800B for an l-major layout); the
    # host untangles the layout during unshard
    out = nc.dram_tensor("out", (N_MS, LCHUNK, N_LC * CHUNK_F), BF16,
                         kind="ExternalOutput").ap()

    WVA = C + 2 * CHUNK_F        # w4t2 + v3 blocks 0-1

    with tile.TileContext(nc) as tc, ExitStack() as ctx:
      const = ctx.enter_context(tc.tile_pool(name="const", bufs=1))
      dpool = ctx.enter_context(tc.tile_pool(name="dpool", bufs=128))
      opool = ctx.enter_context(tc.tile_pool(name="opool", bufs=10))
      psum = ctx.enter_context(tc.tile_pool(name="psum", bufs=4, space="PSUM"))
      for _rep in range(repeat):
        # ---- input loads ----
        # DMA queues are sync/scalar/gpsimd only. A DMA costs ~1.3us
        # fixed startup + transfer at ~230GB/s for >=1KB rows, and a
        # ring's SECOND DMA starts ~1us after its first finishes. So
        # everything D-gen needs rides FIRST on a ring: xin halves
        # (partition split) on sync+scalar, w4t2+v3[0:2] on gpsimd.
        # The scalar ring processes its half while the ACT engine loads
        # the activation table (act_warm), costing ACT nothing.
        xin_tile = const.tile([H, L + 2 * MB], BF16)
        x1t_bf = xin_tile[:, 0:L]
        negx2_f = xin_tile[:, L:].bitcast(F32)
        wv3_tile = const.tile([H, C + MB * C], BF16)
        w4t2_bf = wv3_tile[:, 0:C]
        v3_bf = wv3_tile[:, C:]
        ones_bf = const.tile([1, LCHUNK], BF16)

        nc.scalar.dma_start(xin_tile[64:128, :], xin[64:128, :])
        nc.sync.dma_start(xin_tile[0:64, :], xin[0:64, :])
        nc.gpsimd.dma_start(wv3_tile[:, 0:WVA], wv3[:, 0:WVA])
        nc.vector.memset(ones_bf[:], 1.0)
        # preload the ACT activation table off the critical path
        act_warm = const.tile([1, LCHUNK], BF16)
        nc.scalar.activation(act_warm[:], ones_bf[:],
                             mybir.ActivationFunctionType.Relu)

        # ---- main loop over m-blocks ----
        # Drains emitted one block late so they queue behind the next
        # block's D work in ACT's queue, prioritizing D production.
        pend = None

        def emit_drain(p):
            # psa3_/psb3_ are SEPARATE psum tiles: cross-engine readers of
            # one tile serialize in the framework, so the final block's
            # DVE+ACT halves only run in parallel with two tiles.
            ms_, psa3_, psb3_, last = p
            o_sb = opool.tile([LCHUNK, N_LC * CHUNK_F], BF16)
            o3 = o_sb[:].rearrange("p (lc c) -> p lc c", c=CHUNK_F)
            if not last:
                nc.scalar.copy(o3[:, 0:2], psa3_)
                nc.scalar.copy(o3[:, 2:4], psb3_)
                nc.sync.dma_start(out[ms_], o_sb[:])
            else:  # split engines + 2 DMAs on 2 queues for a short tail
                nc.vector.tensor_copy(o3[:, 0:2], psa3_)
                nc.sync.dma_start(out[ms_, :, 0 : 2 * CHUNK_F],
                                  o_sb[:, 0 : 2 * CHUNK_F])
                nc.scalar.copy(o3[:, 2:4], psb3_)
                nc.scalar.dma_start(out[ms_, :, 2 * CHUNK_F :],
                                    o_sb[:, 2 * CHUNK_F :])

        for ms in range(N_MS):
            # D tiles for this block
            dts = []
            pat = D_PATS[D_NV[ms]]
            for j in range(MSUB):
                m = ms * MSUB + j
                dt_ = dpool.tile([H, L], BF16, tag="d")
                if pat[j] == "V":
                    nc.vector.tensor_scalar(
                        dt_[:], x1t_bf, negx2_f[:, m : m + 1], 0.0,
                        op0=mybir.AluOpType.add, op1=mybir.AluOpType.max)
                else:
                    nc.scalar.activation(
                        dt_[:], x1t_bf, mybir.ActivationFunctionType.Relu,
                        bias=negx2_f[:, m : m + 1], scale=1.0)
                dts.append(dt_)

            if ms == 0:
                # v3 bulk rides second on the gpsimd ring; lands ~15.5us,
                # needed by block 2's t3 (~17.5us)
                nc.gpsimd.dma_start(wv3_tile[:, WVA:], wv3[:, WVA:])
            if pend is not None:
                emit_drain(pend)

            ps_a = psum.tile([LCHUNK, 2 * PS_STRIDE], F32, tag="ps")
            ps_b = psum.tile([LCHUNK, 2 * PS_STRIDE], F32, tag="ps")
            pss = [ps_a, ps_a, ps_b, ps_b]
            # t3 (+t1 fold) opens each group full-width (PSUM zeroing is
            # bank-granular), then the t4s accumulate j-major.
            for lc in range(N_LC):
                nc.tensor.matmul(
                    pss[lc][:, (lc % 2) * PS_STRIDE :
                            (lc % 2) * PS_STRIDE + CHUNK_F],
                    x1t_bf[:, lc * LCHUNK : (lc + 1) * LCHUNK],
                    v3_bf[:, ms * CHUNK_F : (ms + 1) * CHUNK_F],
                    start=True, stop=False, skip_group_check=True)
            for j in range(MSUB):
                for lc in range(N_LC):
                    base = (lc % 2) * PS_STRIDE
                    nc.tensor.matmul(
                        pss[lc][:, base + j * C : base + (j + 1) * C],
                        dts[j][:, lc * LCHUNK : (lc + 1) * LCHUNK],
                        w4t2_bf[:],
                        start=False, stop=(j == MSUB - 1),
                        skip_group_check=True)

            psa3 = ps_a[:].rearrange("p (lc x) -> p lc x",
                                     x=PS_STRIDE)[:, :, 0:CHUNK_F]
            psb3 = ps_b[:].rearrange("p (lc x) -> p lc x",
                                     x=PS_STRIDE)[:, :, 0:CHUNK_F]
            pend = (ms, psa3, psb3, ms == N_MS - 1)
        emit_drain(pend)
    return nc


_COMPILED = {}


def _get_compiled():
    if "nc" not in _COMPILED:
        nc = bacc.Bacc("TRN2", target_bir_lowering=False, debug=False,
                       num_devices=N_CORES)
        build_kernel(nc)
        nc.compile()
        _COMPILED["nc"] = nc
    return _COMPILED["nc"]


def make_in_maps(x1, x2, W, b):
    W1, W2, W3, W4 = (W[:, 0:H], W[:, H : 2 * H], W[:, 2 * H : 3 * H],
                      W[:, 3 * H : 4 * H])
    w13 = (W1 - W4).T.astype(np.float32)          # [H, C]
    w3t = W3.T.astype(np.float32)                 # [H, C]
    w4t2 = (2.0 * W4).T.astype(np.float32)        # [H, C]
    in_maps = []
    for cid in range(N_CORES):
        bb, mblk = cid // 4, cid % 4
        m0 = mblk * MB
        x2blk = x2[bb, m0 : m0 + MB]              # [MB, H]
        x2t = x2blk.T                             # [H, MB]
        # V3[h, m*C+c] = x2t[h,m]*W3T[h,c] + (W1-W4)T[h,c]
        v3 = x2t[:, :, None] * w3t[:, None, :] + w13[:, None, :]
        wv3 = np.concatenate([w4t2, v3.reshape(H, MB * C)], axis=1)
        # xin: x1t bf16 cols 0:512 | negx2 f32 bytes viewed as bf16 cols
        # 512:768 (bitcast back to f32 on device)
        xin = np.empty((H, L + 2 * MB), dtype=BF16_NP)
        xin[:, 0:L] = x1[bb].T.astype(BF16_NP)
        negx2 = np.ascontiguousarray(-x2t.astype(np.float32))
        xin[:, L:] = negx2.view(np.uint16).view(BF16_NP).reshape(H, 2 * MB)
        in_maps.append({
            "xin": xin,
            "wv3": np.ascontiguousarray(wv3.astype(BF16_NP)),
        })
    return in_maps


def t2_bias(x2, W, b):
    """Host-side t2 term: x2 @ (W2+W4).T + bias, [B, L, C] f32."""
    W2 = W[:, H : 2 * H]
    W4 = W[:, 3 * H : 4 * H]
    return (x2 @ (W2 + W4).T + b).astype(np.float32)


def run_on_device(x1, x2, W, b, trace=False, trace_kwargs=None):
    nc = _get_compiled()
    in_maps = make_in_maps(x1, x2, W, b)
    res = bass_utils.run_bass_kernel_spmd(
        nc, in_maps, core_ids=list(range(N_CORES)), trace=trace,
        **(trace_kwargs or {}))
    t2 = t2_bias(x2, W, b)                        # [B, L, C]
    full = np.empty((B, L, L, C), dtype=np.float32)
    for cid in range(N_CORES):
        bb, mblk = cid // 4, cid % 4
        m0 = mblk * MB
        # device out is [ms, p, (lc, j, c)]; l = lc*128+p, m = ms*16+j
        dev = (np.asarray(res.results[cid]["out"])
               .reshape(N_MS, LCHUNK, N_LC, MSUB, C)
               .transpose(2, 1, 0, 3, 4)
               .reshape(L, MB, C).astype(np.float32))
        full[bb, :, m0 : m0 + MB, :] = (
            dev + t2[bb, m0 : m0 + MB, :][None, :, :])
    return full, res


def kernel(x1, x2, W, b):
    x1 = np.asarray(x1, dtype=np.float32)
    x2 = np.asarray(x2, dtype=np.float32)
    W = np.asarray(W, dtype=np.float32)
    b = np.asarray(b, dtype=np.float32)
    full, _ = run_on_device(x1, x2, W, b, trace=False)
    return full
